# revision 33
# baseline (speedup 1.0000x reference)
"""Multi-head attention (B=4, S=2048, E=1024, H=16, hd=64) on 8 TRN2 cores.

Sharding: core c -> batch b = c//2, head-half hh = c%2 (8 heads = 512 internal
dims).  Data parallel on B, tensor parallel on heads.  Each core computes a
partial out-projection for its batch; the host sums the two head-half partials
per batch and adds the (folded) output bias.

Device dataflow (bf16 matmuls, fp32 PSUM accumulation):
  - host pre-transposes q/k/v to (E, S) and casts to bf16 so the projection
    matmuls need no on-chip transpose:
      qhT (512 x 2048) = Wq_loc^T @ qT   [internal dims on partitions]
      khT likewise; vh (2048 x 512+ones) via lhsT = vT slices, rhs = Wv_loc.
  - attention per head-PAIR (2g, 2g+1) per 512-query chunk: row-group
    concurrent K=64 scoresT matmuls for both heads into one PSUM tile, one
    Exp over both (scale 1/8 pre-folded into qhT), then M=65 AV matmuls
    whose ones-column accumulates the softmax denominator in row 64.
  - division: AV promptly evacuated PSUM->SBUF (frees the accumulator),
    denominator row DMA round-trips through DRAM to broadcast across
    partitions, DVE fast-reciprocal + multiply into attn_outT -- exactly the
    lhsT needed for the out-projection po (q x E) = attn_outT^T @ Wo_loc.
  - engines run their streams in order, so projection m-tile g+1 matmuls are
    explicitly interleaved into attention pair g's steps (and out-projection
    into pair 3's) to keep TensorE busy while ScalarE paces the exps.
"""

import math
import sys
from contextlib import ExitStack

sys.path.insert(0, "/opt/trn_rl_repo")

import numpy as np
import ml_dtypes

import concourse.bass as bass
from concourse import bacc
import concourse.mybir as mybir
import concourse.tile as tile

F32 = mybir.dt.float32
BF16 = mybir.dt.bfloat16
AF = mybir.ActivationFunctionType
ALU = mybir.AluOpType

B, S, E = 4, 2048, 1024
H, HD = 16, 64
HLOC = 8          # heads per core
ILOC = HLOC * HD  # 512 internal dims per core
KT = E // 128     # 8 embed k-tiles
ST = S // 128     # 16 seq tiles
NCORES = 8
SCALE = 1.0 / math.sqrt(HD)  # 1/8


def build_nc():
    nc = bacc.Bacc()

    qT_d = nc.declare_dram_parameter("qT", [E, S], BF16, isOutput=False).ap()
    kT_d = nc.declare_dram_parameter("kT", [E, S], BF16, isOutput=False).ap()
    vT_d = nc.declare_dram_parameter("vT", [E, S], BF16, isOutput=False).ap()
    wq_d = nc.declare_dram_parameter("wq", [E, ILOC], BF16, isOutput=False).ap()
    wk_d = nc.declare_dram_parameter("wk", [E, ILOC], BF16, isOutput=False).ap()
    wv_d = nc.declare_dram_parameter("wv", [E, ILOC], BF16, isOutput=False).ap()
    wo_d = nc.declare_dram_parameter("wo", [ILOC, E], BF16, isOutput=False).ap()
    bq_d = nc.declare_dram_parameter("bq", [128, 4], F32, isOutput=False).ap()
    bk_d = nc.declare_dram_parameter("bk", [128, 4], F32, isOutput=False).ap()
    out_d = nc.declare_dram_parameter("out", [S, E], F32, isOutput=True).ap()
    dscr = nc.dram_tensor("dscratch", [32, 512], F32).ap()

    with tile.TileContext(nc) as tc, ExitStack() as ctx:
        # ---- pools (PSUM: pp 2x1 + sc 2x2 + av 2x1 = 8 banks) ----
        psum = ctx.enter_context(tc.tile_pool(name="psum", bufs=2, space="PSUM"))
        av_pool = ctx.enter_context(tc.tile_pool(name="avp", bufs=2, space="PSUM"))
        qhT_pool = ctx.enter_context(tc.tile_pool(name="qhT", bufs=4))
        khT_pool = ctx.enter_context(tc.tile_pool(name="khT", bufs=4))
        vh_pool = ctx.enter_context(tc.tile_pool(name="vh", bufs=ST))
        bias_pool = ctx.enter_context(tc.tile_pool(name="bias", bufs=1))
        wpool = ctx.enter_context(tc.tile_pool(name="w_in", bufs=4))
        stage_pool = ctx.enter_context(tc.tile_pool(name="stage", bufs=17))
        exp_pool = ctx.enter_context(tc.tile_pool(name="exp", bufs=4))
        attnT_pool = ctx.enter_context(tc.tile_pool(name="attnT", bufs=4))
        small_pool = ctx.enter_context(tc.tile_pool(name="small", bufs=4))
        bc_pool = ctx.enter_context(tc.tile_pool(name="bcb", bufs=4))
        tmp_pool = ctx.enter_context(tc.tile_pool(name="tmpp", bufs=2))
        out_pool = ctx.enter_context(tc.tile_pool(name="outbuf", bufs=2))

        qhT = [qhT_pool.tile([128, S], BF16, tag="qhT", name=f"qhT{i}")
               for i in range(4)]
        khT = [khT_pool.tile([128, S], BF16, tag="khT", name=f"khT{i}")
               for i in range(4)]
        vh = [vh_pool.tile([128, HLOC * 65], BF16, tag="vh", name=f"vh{i}")
              for i in range(ST)]

        bq_t = bias_pool.tile([128, 4], F32, tag="bq")
        bk_t = bias_pool.tile([128, 4], F32, tag="bk")
        nc.sync.dma_start(bq_t[:], bq_d[:])
        nc.sync.dma_start(bk_t[:], bk_d[:])

        wq_t = wpool.tile([128, KT, ILOC], BF16, tag="w")
        wk_t = wpool.tile([128, KT, ILOC], BF16, tag="w")
        wv_t = wpool.tile([128, KT, ILOC], BF16, tag="w")
        wo_t = wpool.tile([128, 4, E], BF16, tag="w")
        attnT = [attnT_pool.tile([128, S], BF16, tag="attnT",
                                 name=f"attnT{i}") for i in range(4)]

        # ---- weight + stage loads, v first so vh proj starts earliest ----
        stg_v, stg_k, stg_q = [], [], []
        nc.sync.dma_start(wv_t[:], wv_d.rearrange("(k p) n -> p k n", p=128))
        for (stg, src_d), w_dma in (
            ((stg_v, vT_d), lambda: nc.sync.dma_start(
                wk_t[:], wk_d.rearrange("(k p) n -> p k n", p=128))),
            ((stg_k, kT_d), lambda: nc.sync.dma_start(
                wq_t[:], wq_d.rearrange("(k p) n -> p k n", p=128))),
            ((stg_q, qT_d), lambda: nc.sync.dma_start(
                wo_t[:], wo_d.rearrange("(i p) n -> p i n", p=128))),
        ):
            for kk in range(KT):
                t = stage_pool.tile([128, S], BF16, tag="stage")
                nc.sync.dma_start(t[:], src_d[kk * 128:(kk + 1) * 128, :])
                stg.append(t)
            w_dma()

        # ---- vh projection (attention needs all of it up front) ----
        for st in range(ST):
            ps = psum.tile([128, 512], F32, tag="pp", name="psv")
            for kk in range(KT):
                nc.tensor.matmul(
                    ps[:],
                    lhsT=stg_v[kk][:, st * 128:(st + 1) * 128],
                    rhs=wv_t[:, kk, :],
                    start=(kk == 0), stop=(kk == KT - 1),
                )
            pin = ps[:].rearrange("p (a b x) -> p a b x", b=2, x=64)
            pout = vh[st].rearrange("p (a c) -> p a c", c=130)
            nc.vector.tensor_copy(pout[:, :, 0:64], pin[:, :, 0, :])
            nc.vector.tensor_copy(pout[:, :, 65:129], pin[:, :, 1, :])
            ones = vh[st].rearrange("p (h x) -> p h x", x=65)[:, :, 64:65]
            nc.vector.memset(ones, 1.0)

        def proj_ops(m, which="kq", quarters=range(4)):
            """Closure list projecting khT/qhT m-tile chains."""
            ops = []
            srcs = {"k": ((stg_k, wk_t, khT, bk_t, 1.0),),
                    "q": ((stg_q, wq_t, qhT, bq_t, SCALE),)}
            chosen = srcs["k"] + srcs["q"] if which == "kq" else srcs[which]
            for stg, w_t, dst, b_t, scale in chosen:
                for quarter in quarters:
                    cols = slice(quarter * 512, (quarter + 1) * 512)
                    holder = {}

                    for kk in range(KT):
                        def mm(kk=kk, stg=stg, w_t=w_t, cols=cols,
                               holder=holder, first=(kk == 0)):
                            if first:
                                holder["ps"] = psum.tile(
                                    [128, 512], F32, tag="pp", name="psqk")
                            nc.tensor.matmul(
                                holder["ps"][:],
                                lhsT=w_t[:, kk, m * 128:(m + 1) * 128],
                                rhs=stg[kk][:, cols],
                                start=(kk == 0), stop=(kk == KT - 1),
                            )
                        ops.append(mm)

                    def evac(dst=dst, cols=cols, b_t=b_t, scale=scale, m=m,
                             holder=holder):
                        nc.scalar.activation(
                            dst[m][:, cols], holder["ps"][:], AF.Identity,
                            bias=b_t[:, m:m + 1], scale=scale,
                        )
                    ops.append(evac)
            return ops

        def outproj_ops(qc):
            """Closure list projecting output for query chunk qc."""
            ops = []
            for qt in range(qc * 4, qc * 4 + 4):
                holder = {}

                for c in range(2):
                    for it in range(4):
                        def mm(qt=qt, c=c, it=it, holder=holder,
                               first=(c == 0 and it == 0)):
                            if first:
                                holder["ot"] = out_pool.tile(
                                    [128, 1024], F32, tag="ot", name="ot")
                            if it == 0:
                                holder["po"] = psum.tile(
                                    [128, 512], F32, tag="pp", name="po")
                            nc.tensor.matmul(
                                holder["po"][:],
                                lhsT=attnT[it][:, qt * 128:(qt + 1) * 128],
                                rhs=wo_t[:, it, c * 512:(c + 1) * 512],
                                start=(it == 0), stop=(it == 3),
                            )
                        ops.append(mm)

                    def evac(qt=qt, c=c, holder=holder, last=(c == 1)):
                        nc.vector.tensor_copy(
                            holder["ot"][:, c * 512:(c + 1) * 512],
                            holder["po"][:])
                        if last:
                            nc.sync.dma_start(
                                out_d[qt * 128:(qt + 1) * 128, :],
                                holder["ot"][:])
                    ops.append(evac)
            return ops

        # upfront: k m=0 fully + q m=0 quarter 0; the rest interleaves
        for op in proj_ops(0, "k"):
            op()
        for op in proj_ops(0, "q", quarters=(0,)):
            op()

        for g in range(4):              # head pair (2g, 2g+1)
            hA, hB = 2 * g, 2 * g + 1
            if g == 0:
                fillers = proj_ops(0, "q", quarters=(1, 2, 3)) + proj_ops(1)
            elif g < 3:
                fillers = proj_ops(g + 1)
            else:
                fillers = []
            steps_left = 4 * ST
            for qc in range(4):         # 512-query chunks
                if g == 3 and qc >= 1:
                    fillers.extend(outproj_ops(qc - 1))
                qcols = slice(qc * 512, (qc + 1) * 512)
                avA = av_pool.tile([65, 512], F32, tag="av", name="avA")
                avB = av_pool.tile([65, 512], F32, tag="av", name="avB")
                for kt in range(ST):
                    sc = psum.tile([128, 1024], F32, tag="sc", name="sc")
                    nc.tensor.matmul(
                        sc[:, 0:512],
                        lhsT=khT[g][0:64, kt * 128:(kt + 1) * 128],
                        rhs=qhT[g][0:64, qcols],
                        start=True, stop=True,
                    )
                    nc.tensor.matmul(
                        sc[:, 512:1024],
                        lhsT=khT[g][64:128, kt * 128:(kt + 1) * 128],
                        rhs=qhT[g][64:128, qcols],
                        start=True, stop=True,
                    )
                    ex = exp_pool.tile([128, 1024], BF16, tag="exp", name="ex")
                    nc.scalar.activation(ex[:], sc[:], AF.Exp)
                    first, last = (kt == 0), (kt == ST - 1)
                    nc.tensor.matmul(
                        avA[0:65, :],
                        lhsT=vh[kt][:, hA * 65:hA * 65 + 65],
                        rhs=ex[:, 0:512],
                        start=first, stop=last,
                    )
                    nc.tensor.matmul(
                        avB[0:65, :],
                        lhsT=vh[kt][:, hB * 65:hB * 65 + 65],
                        rhs=ex[:, 512:1024],
                        start=first, stop=last,
                    )
                    # pace interleaved filler work (proj / out-proj)
                    steps_left -= 1
                    n_take = -(-len(fillers) // max(steps_left, 1)) \
                        if fillers else 0
                    for _ in range(min(n_take, len(fillers))):
                        fillers.pop(0)()
                # ---- softmax division (prompt PSUM evacuation) ----
                idx = (g * 4 + qc) * 2
                avsA = small_pool.tile([65, 512], F32, tag="avs", name="avsA")
                avsB = small_pool.tile([65, 512], F32, tag="avs", name="avsB")
                nc.vector.tensor_copy(avsA[:], avA[:])
                nc.vector.tensor_copy(avsB[:], avB[:])
                nc.sync.dma_start(dscr[idx:idx + 1, :], avsA[64:65, :])
                nc.sync.dma_start(dscr[idx + 1:idx + 2, :], avsB[64:65, :])
                bcA = bc_pool.tile([64, 512], F32, tag="bc", name="bcA")
                bcB = bc_pool.tile([64, 512], F32, tag="bc", name="bcB")
                nc.sync.dma_start(
                    bcA[:].rearrange("p (o n) -> p o n", o=1),
                    dscr[idx, :].partition_broadcast(64))
                nc.sync.dma_start(
                    bcB[:].rearrange("p (o n) -> p o n", o=1),
                    dscr[idx + 1, :].partition_broadcast(64))
                nc.vector.reciprocal_approx_fast(bcA[:], bcA[:])
                nc.vector.reciprocal_approx_fast(bcB[:], bcB[:])
                nc.vector.tensor_mul(attnT[g][0:64, qcols],
                                     avsA[0:64, :], bcA[:])
                tmp = tmp_pool.tile([64, 512], BF16, tag="tmp", name="tmp")
                nc.vector.tensor_mul(tmp[:], avsB[0:64, :], bcB[:])
                nc.sync.dma_start(attnT[g][64:128, qcols], tmp[:])
            # flush any leftover fillers for this pair
            for op in fillers:
                op()

        # final out-projection chunk
        for op in outproj_ops(3):
            op()

    nc.finalize()
    return nc


def make_in_maps(q, k, v, Wq, bq, Wk, bk, Wv, bv, Wo, bo):
    """Per-core input dicts + the folded host-side bias."""
    bf = ml_dtypes.bfloat16
    qT = [np.ascontiguousarray(q[b].T).astype(bf) for b in range(B)]
    kT = [np.ascontiguousarray(k[b].T).astype(bf) for b in range(B)]
    vT = [np.ascontiguousarray(v[b].T).astype(bf) for b in range(B)]
    in_maps = []
    for c in range(NCORES):
        b, hh = divmod(c, 2)
        isl = slice(hh * ILOC, (hh + 1) * ILOC)
        bq_loc = np.ascontiguousarray(
            (bq[isl] * SCALE).reshape(4, 128).T)
        bk_loc = np.ascontiguousarray(bk[isl].reshape(4, 128).T)
        in_maps.append({
            "qT": qT[b], "kT": kT[b], "vT": vT[b],
            "wq": np.ascontiguousarray(Wq[:, isl]).astype(bf),
            "wk": np.ascontiguousarray(Wk[:, isl]).astype(bf),
            "wv": np.ascontiguousarray(Wv[:, isl]).astype(bf),
            "wo": np.ascontiguousarray(Wo[isl, :]).astype(bf),
            "bq": bq_loc, "bk": bk_loc,
        })
    bo_eff = (bo + bv @ Wo).astype(np.float32)
    return in_maps, bo_eff


_NC_CACHE = None


def kernel(q, k, v, Wq, bq, Wk, bk, Wv, bv, Wo, bo):
    global _NC_CACHE
    from concourse.bass_utils import run_bass_kernel_spmd

    if _NC_CACHE is None:
        _NC_CACHE = build_nc()
    nc = _NC_CACHE
    in_maps, bo_eff = make_in_maps(q, k, v, Wq, bq, Wk, bk, Wv, bv, Wo, bo)
    res = run_bass_kernel_spmd(nc, in_maps, list(range(NCORES)))
    out = np.empty((B, S, E), np.float32)
    for b in range(B):
        out[b] = res.results[2 * b]["out"] + res.results[2 * b + 1]["out"] + bo_eff
    return out


# revision 35
# speedup vs baseline: 1.0045x; 1.0045x over previous
"""Multi-head attention (B=4, S=2048, E=1024, H=16, hd=64) on 8 TRN2 cores.

Sharding: core c -> batch b = c//2, head-half hh = c%2 (8 heads = 512 internal
dims).  Data parallel on B, tensor parallel on heads.  Each core computes a
partial out-projection for its batch; the host sums the two head-half partials
per batch and adds the (folded) output bias.

Device dataflow (bf16 matmuls, fp32 PSUM accumulation):
  - host pre-transposes q/k/v to (E, S) and casts to bf16 so the projection
    matmuls need no on-chip transpose:
      qhT (512 x 2048) = Wq_loc^T @ qT   [internal dims on partitions]
      khT likewise; vh (2048 x 512+ones) via lhsT = vT slices, rhs = Wv_loc.
  - attention per head-PAIR (2g, 2g+1) per 512-query chunk: row-group
    concurrent K=64 scoresT matmuls for both heads into one PSUM tile, one
    Exp over both (scale 1/8 pre-folded into qhT), then M=65 AV matmuls
    whose ones-column accumulates the softmax denominator in row 64.
  - division: AV promptly evacuated PSUM->SBUF (frees the accumulator),
    denominator row DMA round-trips through DRAM to broadcast across
    partitions, DVE fast-reciprocal + multiply into attn_outT -- exactly the
    lhsT needed for the out-projection po (q x E) = attn_outT^T @ Wo_loc.
  - engines run their streams in order, so projection m-tile g+1 matmuls are
    explicitly interleaved into attention pair g's steps (and out-projection
    into pair 3's) to keep TensorE busy while ScalarE paces the exps.
"""

import math
import sys
from contextlib import ExitStack

sys.path.insert(0, "/opt/trn_rl_repo")

import numpy as np
import ml_dtypes

import concourse.bass as bass
from concourse import bacc
import concourse.mybir as mybir
import concourse.tile as tile

F32 = mybir.dt.float32
BF16 = mybir.dt.bfloat16
AF = mybir.ActivationFunctionType
ALU = mybir.AluOpType

B, S, E = 4, 2048, 1024
H, HD = 16, 64
HLOC = 8          # heads per core
ILOC = HLOC * HD  # 512 internal dims per core
KT = E // 128     # 8 embed k-tiles
ST = S // 128     # 16 seq tiles
NCORES = 8
SCALE = 1.0 / math.sqrt(HD)  # 1/8


def build_nc():
    nc = bacc.Bacc()

    qT_d = nc.declare_dram_parameter("qT", [E, S], BF16, isOutput=False).ap()
    kT_d = nc.declare_dram_parameter("kT", [E, S], BF16, isOutput=False).ap()
    vT_d = nc.declare_dram_parameter("vT", [E, S], BF16, isOutput=False).ap()
    wq_d = nc.declare_dram_parameter("wq", [E, ILOC], BF16, isOutput=False).ap()
    wk_d = nc.declare_dram_parameter("wk", [E, ILOC], BF16, isOutput=False).ap()
    wv_d = nc.declare_dram_parameter("wv", [E, ILOC], BF16, isOutput=False).ap()
    wo_d = nc.declare_dram_parameter("wo", [ILOC, E], BF16, isOutput=False).ap()
    bq_d = nc.declare_dram_parameter("bq", [128, 4], F32, isOutput=False).ap()
    bk_d = nc.declare_dram_parameter("bk", [128, 4], F32, isOutput=False).ap()
    out_d = nc.declare_dram_parameter("out", [S, E], F32, isOutput=True).ap()
    dscr = nc.dram_tensor("dscratch", [32, 512], F32).ap()

    with tile.TileContext(nc) as tc, ExitStack() as ctx:
        # ---- pools (PSUM: pp 2x1 + sc 2x2 + av 2x1 = 8 banks) ----
        psum = ctx.enter_context(tc.tile_pool(name="psum", bufs=2, space="PSUM"))
        av_pool = ctx.enter_context(tc.tile_pool(name="avp", bufs=2, space="PSUM"))
        qhT_pool = ctx.enter_context(tc.tile_pool(name="qhT", bufs=4))
        khT_pool = ctx.enter_context(tc.tile_pool(name="khT", bufs=4))
        vh_pool = ctx.enter_context(tc.tile_pool(name="vh", bufs=ST))
        bias_pool = ctx.enter_context(tc.tile_pool(name="bias", bufs=1))
        wpool = ctx.enter_context(tc.tile_pool(name="w_in", bufs=4))
        stage_pool = ctx.enter_context(tc.tile_pool(name="stage", bufs=17))
        exp_pool = ctx.enter_context(tc.tile_pool(name="exp", bufs=4))
        attnT_pool = ctx.enter_context(tc.tile_pool(name="attnT", bufs=4))
        small_pool = ctx.enter_context(tc.tile_pool(name="small", bufs=4))
        bc_pool = ctx.enter_context(tc.tile_pool(name="bcb", bufs=4))
        tmp_pool = ctx.enter_context(tc.tile_pool(name="tmpp", bufs=2))
        out_pool = ctx.enter_context(tc.tile_pool(name="outbuf", bufs=2))

        qhT = [qhT_pool.tile([128, S], BF16, tag="qhT", name=f"qhT{i}")
               for i in range(4)]
        khT = [khT_pool.tile([128, S], BF16, tag="khT", name=f"khT{i}")
               for i in range(4)]
        vh = [vh_pool.tile([128, HLOC * 65], BF16, tag="vh", name=f"vh{i}")
              for i in range(ST)]

        bq_t = bias_pool.tile([128, 4], F32, tag="bq")
        bk_t = bias_pool.tile([128, 4], F32, tag="bk")
        nc.sync.dma_start(bq_t[:], bq_d[:])
        nc.sync.dma_start(bk_t[:], bk_d[:])

        wq_t = wpool.tile([128, KT, ILOC], BF16, tag="w")
        wk_t = wpool.tile([128, KT, ILOC], BF16, tag="w")
        wv_t = wpool.tile([128, KT, ILOC], BF16, tag="w")
        wo_t = wpool.tile([128, 4, E], BF16, tag="w")
        attnT = [attnT_pool.tile([128, S], BF16, tag="attnT",
                                 name=f"attnT{i}") for i in range(4)]

        # ---- weight + stage loads, v first so vh proj starts earliest ----
        stg_v, stg_k, stg_q = [], [], []
        nc.sync.dma_start(wv_t[:], wv_d.rearrange("(k p) n -> p k n", p=128))
        for (stg, src_d), w_dma in (
            ((stg_v, vT_d), lambda: nc.sync.dma_start(
                wk_t[:], wk_d.rearrange("(k p) n -> p k n", p=128))),
            ((stg_k, kT_d), lambda: nc.sync.dma_start(
                wq_t[:], wq_d.rearrange("(k p) n -> p k n", p=128))),
            ((stg_q, qT_d), lambda: nc.sync.dma_start(
                wo_t[:], wo_d.rearrange("(i p) n -> p i n", p=128))),
        ):
            for kk in range(KT):
                t = stage_pool.tile([128, S], BF16, tag="stage")
                nc.sync.dma_start(t[:], src_d[kk * 128:(kk + 1) * 128, :])
                stg.append(t)
            w_dma()

        # ---- vh projection (attention needs all of it up front) ----
        for st in range(ST):
            ps = psum.tile([128, 512], F32, tag="pp", name="psv")
            for kk in range(KT):
                nc.tensor.matmul(
                    ps[:],
                    lhsT=stg_v[kk][:, st * 128:(st + 1) * 128],
                    rhs=wv_t[:, kk, :],
                    start=(kk == 0), stop=(kk == KT - 1),
                )
            pin = ps[:].rearrange("p (a b x) -> p a b x", b=2, x=64)
            pout = vh[st].rearrange("p (a c) -> p a c", c=130)
            nc.vector.tensor_copy(pout[:, :, 0:64], pin[:, :, 0, :])
            nc.vector.tensor_copy(pout[:, :, 65:129], pin[:, :, 1, :])
            ones = vh[st].rearrange("p (h x) -> p h x", x=65)[:, :, 64:65]
            nc.vector.memset(ones, 1.0)

        def proj_ops(m, which="kq", quarters=range(4)):
            """Closure list projecting khT/qhT m-tile chains."""
            ops = []
            srcs = {"k": ((stg_k, wk_t, khT, bk_t, 1.0),),
                    "q": ((stg_q, wq_t, qhT, bq_t, SCALE),)}
            chosen = srcs["k"] + srcs["q"] if which == "kq" else srcs[which]
            for stg, w_t, dst, b_t, scale in chosen:
                for quarter in quarters:
                    cols = slice(quarter * 512, (quarter + 1) * 512)
                    holder = {}

                    for kk in range(KT):
                        def mm(kk=kk, stg=stg, w_t=w_t, cols=cols,
                               holder=holder, first=(kk == 0)):
                            if first:
                                holder["ps"] = psum.tile(
                                    [128, 512], F32, tag="pp", name="psqk")
                            nc.tensor.matmul(
                                holder["ps"][:],
                                lhsT=w_t[:, kk, m * 128:(m + 1) * 128],
                                rhs=stg[kk][:, cols],
                                start=(kk == 0), stop=(kk == KT - 1),
                            )
                        ops.append(mm)

                    def evac(dst=dst, cols=cols, b_t=b_t, scale=scale, m=m,
                             holder=holder):
                        nc.scalar.activation(
                            dst[m][:, cols], holder["ps"][:], AF.Identity,
                            bias=b_t[:, m:m + 1], scale=scale,
                        )
                    ops.append(evac)
            return ops

        def outproj_ops(qc):
            """Closure list projecting output for query chunk qc."""
            ops = []
            for qt in range(qc * 4, qc * 4 + 4):
                holder = {}

                for c in range(2):
                    for it in range(4):
                        def mm(qt=qt, c=c, it=it, holder=holder,
                               first=(c == 0 and it == 0)):
                            if first:
                                holder["ot"] = out_pool.tile(
                                    [128, 1024], F32, tag="ot", name="ot")
                            if it == 0:
                                holder["po"] = psum.tile(
                                    [128, 512], F32, tag="pp", name="po")
                            nc.tensor.matmul(
                                holder["po"][:],
                                lhsT=attnT[it][:, qt * 128:(qt + 1) * 128],
                                rhs=wo_t[:, it, c * 512:(c + 1) * 512],
                                start=(it == 0), stop=(it == 3),
                            )
                        ops.append(mm)

                    def evac(qt=qt, c=c, holder=holder, last=(c == 1)):
                        nc.vector.tensor_copy(
                            holder["ot"][:, c * 512:(c + 1) * 512],
                            holder["po"][:])
                        if last:
                            nc.sync.dma_start(
                                out_d[qt * 128:(qt + 1) * 128, :],
                                holder["ot"][:])
                    ops.append(evac)
            return ops

        # upfront: k m=0 fully + q m=0 quarter 0; the rest interleaves
        for op in proj_ops(0, "k"):
            op()
        for op in proj_ops(0, "q", quarters=(0,)):
            op()

        for g in range(4):              # head pair (2g, 2g+1)
            hA, hB = 2 * g, 2 * g + 1
            if g == 0:
                fillers = proj_ops(0, "q", quarters=(1, 2, 3)) + proj_ops(1)
            elif g < 3:
                fillers = proj_ops(g + 1)
            else:
                fillers = []
            steps_left = 4 * ST
            for qc in range(4):         # 512-query chunks
                if g == 3 and qc >= 1:
                    fillers.extend(outproj_ops(qc - 1))
                qcols = slice(qc * 512, (qc + 1) * 512)
                avA = av_pool.tile([65, 512], F32, tag="av", name="avA")
                avB = av_pool.tile([65, 512], F32, tag="av", name="avB")
                for kt in range(ST):
                    sc = psum.tile([128, 1024], F32, tag="sc", name="sc")
                    nc.tensor.matmul(
                        sc[:, 0:512],
                        lhsT=khT[g][0:64, kt * 128:(kt + 1) * 128],
                        rhs=qhT[g][0:64, qcols],
                        start=True, stop=True,
                    )
                    nc.tensor.matmul(
                        sc[:, 512:1024],
                        lhsT=khT[g][64:128, kt * 128:(kt + 1) * 128],
                        rhs=qhT[g][64:128, qcols],
                        start=True, stop=True,
                    )
                    ex = exp_pool.tile([128, 1024], BF16, tag="exp", name="ex")
                    nc.scalar.activation(ex[:], sc[:], AF.Exp)
                    first, last = (kt == 0), (kt == ST - 1)
                    nc.tensor.matmul(
                        avA[0:65, :],
                        lhsT=vh[kt][:, hA * 65:hA * 65 + 65],
                        rhs=ex[:, 0:512],
                        start=first, stop=last,
                    )
                    nc.tensor.matmul(
                        avB[0:65, :],
                        lhsT=vh[kt][:, hB * 65:hB * 65 + 65],
                        rhs=ex[:, 512:1024],
                        start=first, stop=last,
                    )
                    # pace interleaved filler work (proj / out-proj)
                    steps_left -= 1
                    n_take = -(-len(fillers) // max(steps_left, 1)) \
                        if fillers else 0
                    for _ in range(min(n_take, len(fillers))):
                        fillers.pop(0)()
                # ---- softmax division (prompt PSUM evacuation) ----
                idx = (g * 4 + qc) * 2
                avsA = small_pool.tile([65, 512], F32, tag="avs", name="avsA")
                avsB = small_pool.tile([65, 512], F32, tag="avs", name="avsB")
                nc.vector.tensor_copy(avsA[:], avA[:])
                nc.vector.tensor_copy(avsB[:], avB[:])
                nc.sync.dma_start(dscr[idx:idx + 1, :], avsA[64:65, :])
                nc.sync.dma_start(dscr[idx + 1:idx + 2, :], avsB[64:65, :])
                bcA = bc_pool.tile([64, 512], F32, tag="bc", name="bcA")
                bcB = bc_pool.tile([64, 512], F32, tag="bc", name="bcB")
                nc.sync.dma_start(
                    bcA[:].rearrange("p (o n) -> p o n", o=1),
                    dscr[idx, :].partition_broadcast(64))
                nc.sync.dma_start(
                    bcB[:].rearrange("p (o n) -> p o n", o=1),
                    dscr[idx + 1, :].partition_broadcast(64))
                nc.vector.reciprocal_approx_fast(bcA[:], bcA[:])
                nc.vector.reciprocal_approx_fast(bcB[:], bcB[:])
                nc.vector.tensor_mul(attnT[g][0:64, qcols],
                                     avsA[0:64, :], bcA[:])
                tmp = tmp_pool.tile([64, 512], BF16, tag="tmp", name="tmp")
                nc.vector.tensor_mul(tmp[:], avsB[0:64, :], bcB[:])
                nc.sync.dma_start(attnT[g][64:128, qcols], tmp[:])
            # flush any leftover fillers for this pair
            for op in fillers:
                op()

        # final out-projection chunk
        for op in outproj_ops(3):
            op()

    nc.finalize()
    return nc


def make_in_maps(q, k, v, Wq, bq, Wk, bk, Wv, bv, Wo, bo):
    """Per-core input dicts + the folded host-side bias."""
    bf = ml_dtypes.bfloat16
    qT = [np.ascontiguousarray(q[b].T).astype(bf) for b in range(B)]
    kT = [np.ascontiguousarray(k[b].T).astype(bf) for b in range(B)]
    vT = [np.ascontiguousarray(v[b].T).astype(bf) for b in range(B)]
    in_maps = []
    for c in range(NCORES):
        b, hh = divmod(c, 2)
        isl = slice(hh * ILOC, (hh + 1) * ILOC)
        bq_loc = np.ascontiguousarray(
            (bq[isl] * SCALE).reshape(4, 128).T)
        bk_loc = np.ascontiguousarray(bk[isl].reshape(4, 128).T)
        in_maps.append({
            "qT": qT[b], "kT": kT[b], "vT": vT[b],
            "wq": np.ascontiguousarray(Wq[:, isl]).astype(bf),
            "wk": np.ascontiguousarray(Wk[:, isl]).astype(bf),
            "wv": np.ascontiguousarray(Wv[:, isl]).astype(bf),
            "wo": np.ascontiguousarray(Wo[isl, :]).astype(bf),
            "bq": bq_loc, "bk": bk_loc,
        })
    bo_eff = (bo + bv @ Wo).astype(np.float32)
    return in_maps, bo_eff


_NC_CACHE = None


def kernel(q, k, v, Wq, bq, Wk, bk, Wv, bv, Wo, bo):
    global _NC_CACHE
    from concourse.bass_utils import run_bass_kernel_spmd

    if _NC_CACHE is None:
        _NC_CACHE = build_nc()
    nc = _NC_CACHE
    in_maps, bo_eff = make_in_maps(q, k, v, Wq, bq, Wk, bk, Wv, bv, Wo, bo)
    res = run_bass_kernel_spmd(nc, in_maps, list(range(NCORES)))
    out = np.empty((B, S, E), np.float32)
    for b in range(B):
        out[b] = res.results[2 * b]["out"] + res.results[2 * b + 1]["out"] + bo_eff
    return out
